# revision 1
# baseline (speedup 1.0000x reference)
"""Trainium2 Bass kernel: AdditiveAttention-style scoring head.

Computes, for x:(B,N,D), W1/W2:(A,D), b1/b2:(A,), Wout:(A,), bout:(1,):
    x1 = x @ W1.T + b1                       (B,N,A)
    x2 = x @ W2.T + b2                       (B,N,A)
    out[b,i-1,j] = sum_a Wout[a]*tanh(x1[b,j,a] + x2[b,i,a]) + bout,  i=1..N-1

Sharding: data-parallel over batch B across 8 NeuronCores (B/8=4 per core),
weights replicated, no collectives. Per core the 33M-element tanh stream is
the roofline (ACT engine, 128 lanes @1.2GHz); the broadcast add runs on DVE
(bf16 2x mode via duplicated-pair APs), and the A-reduction against Wout runs
on the TensorEngine as accumulating K=128 matmuls whose zero-padded
stationary operand routes each 512-col slice to its own PSUM partition
(even/odd slices on two banks so accumulation chains don't serialize); a
fused DVE tensor_scalar stages PSUM->SBUF adding bout on the way out.
"""
import sys
import numpy as np

if "/opt/trn_rl_repo" not in sys.path:
    sys.path.insert(0, "/opt/trn_rl_repo")

B, N, D, A = 32, 128, 512, 512
NCORES = 8
BPC = B // NCORES      # batches per core
KC = D // 128          # contraction chunks for the input matmuls
MC = A // 128          # a-chunks (partition dim of the fused stage)
IB = 64                # i-rows per pipeline block
NIB = N // IB          # i-blocks per batch (covers i=0..N-1; i=0 dropped at DMA)
F = IB * N             # free elements per (b, iblock) tile
MMN = 512              # matmul free dim (one psum bank)
G = F // MMN           # output slices per iblock (8): even/odd across 2 psum banks
GH = G // 2            # slices per bank (4) = psum rows used

_CACHE = {}


def _build_nc():
    import concourse.bass as bass
    import concourse.bacc as bacc
    import concourse.mybir as mybir
    from concourse import tile

    f32 = mybir.dt.float32
    bf16 = mybir.dt.bfloat16
    AF = mybir.ActivationFunctionType

    nc = bacc.Bacc(None, target_bir_lowering=False)

    xT = nc.declare_dram_parameter("xT", [D, BPC * N], bf16, isOutput=False)
    # w?tc[mc, d, j] = W?[mc*128+j, d] — a-chunk-major so chunk 0's weights
    # land first and the main pipeline starts early
    w1t = nc.declare_dram_parameter("w1t", [MC, D, 128], bf16, isOutput=False)
    w2t = nc.declare_dram_parameter("w2t", [MC, D, 128], bf16, isOutput=False)
    b1c = nc.declare_dram_parameter("b1c", [128, MC], f32, isOutput=False)
    b2c = nc.declare_dram_parameter("b2c", [128, MC], f32, isOutput=False)
    # woutpad: per-(c, r) stationary [128, GH] tiles, nonzero only in column r
    # = Wout chunk c. An MM with this lhsT routes its slice's reduction to
    # psum partition r (rows != r accumulate +0). The (c, r) tile is shared
    # by the even and odd slice 2r/2r+1 MMs (two different psum banks).
    woutpad = nc.declare_dram_parameter("woutpad", [128, MC * GH * GH], bf16, isOutput=False)
    boutp = nc.declare_dram_parameter("bout", [128, 1], f32, isOutput=False)
    out = nc.declare_dram_parameter("out", [BPC, (N - 1) * N], f32, isOutput=True)

    with tile.TileContext(nc) as tc:
        with (
            tc.tile_pool(name="const", bufs=1) as cpool,
            tc.tile_pool(name="xw", bufs=1) as xwpool,
            tc.tile_pool(name="x12", bufs=1) as xpool,
            tc.tile_pool(name="s", bufs=3) as spool,
            tc.tile_pool(name="t", bufs=5) as tpool,
            tc.tile_pool(name="stage", bufs=4) as stpool,
        ):
            # ---- PE warmup: dummy matmuls on junk data during the input DMA
            # window so the HAM clock-gate is at 8/8 when the real matmuls
            # arrive (net positive: cold setup matmuls cost more than the
            # warmup's PE-queue occupancy) ----
            warm = cpool.tile([128, MMN], bf16, tag="warm")
            nc.gpsimd.memset(warm[:, :], 0.25)
            with tc.tile_pool(name="psW", bufs=1, space=bass.MemorySpace.PSUM) as psW:
                wps = psW.tile([128, MMN], f32, tag="psW")
                for _ in range(9):
                    nc.tensor.matmul(wps[:, :], warm[:, 0:128], warm[:, :],
                                     start=True, stop=True)

            # ---- input loads (bf16); xT split across two DMA queues, weights
            # in mc-major order on the gpsimd queue so mc=0 lands first ----
            xT_sb = []
            for k in range(KC):
                tx = xwpool.tile([128, BPC * N], bf16, tag=f"xT{k}")
                eng = nc.sync if k % 2 == 0 else nc.scalar
                eng.dma_start(tx[:, :], xT[k * 128:(k + 1) * 128, :])
                xT_sb.append(tx)
            # One 3D-AP DMA per (matrix, a-chunk): SBUF [d', k*128+j] <-
            # DRAM w?t[m, k*128+d', j]; w2 on the gpsimd queue, w1 on scalar.
            w1_sb, w2_sb = [], []
            for m in range(MC):
                t2 = xwpool.tile([128, KC * 128], bf16, tag=f"w2{m}", name=f"w2_{m}")
                d2 = t2[:, :]
                dst2 = bass.AP(d2.tensor, d2.offset,
                               [[d2.ap[0][0], 128], [128, KC], [1, 128]])
                src2 = bass.AP(w2t[0, :, :].tensor, m * D * 128,
                               [[128, 128], [128 * 128, KC], [1, 128]])
                nc.gpsimd.dma_start(dst2, src2)
                w2_sb.append(t2)
                t1 = xwpool.tile([128, KC * 128], bf16, tag=f"w1{m}", name=f"w1_{m}")
                d1 = t1[:, :]
                dst1 = bass.AP(d1.tensor, d1.offset,
                               [[d1.ap[0][0], 128], [128, KC], [1, 128]])
                src1 = bass.AP(w1t[0, :, :].tensor, m * D * 128,
                               [[128, 128], [128 * 128, KC], [1, 128]])
                nc.scalar.dma_start(dst1, src1)
                w1_sb.append(t1)
            b1_sb = cpool.tile([128, MC], f32, tag="b1")
            nc.sync.dma_start(b1_sb[:, :], b1c[:, :])
            b2_sb = cpool.tile([128, MC], f32, tag="b2")
            nc.sync.dma_start(b2_sb[:, :], b2c[:, :])
            wout_sb = cpool.tile([128, MC * GH * GH], bf16, tag="wout")
            nc.sync.dma_start(wout_sb[:, :], woutpad[:, :])
            boutf = cpool.tile([128, 1], f32, tag="boutf")
            nc.sync.dma_start(boutf[:, :], boutp[:, :])

            x1_sb = [xpool.tile([128, BPC * N], bf16, tag=f"x1_{c}", name=f"x1_{c}") for c in range(MC)]
            x2d_sb = [xpool.tile([128, BPC * N * 2], bf16, tag=f"x2d_{c}", name=f"x2d_{c}") for c in range(MC)]

            # ---- x1/x2 = W @ x^T + b, in [a_chunk, (b,n)] layout, cast bf16.
            # Emitted lazily per chunk, interleaved with the first block's
            # TT/ACT ops so the DVE doesn't front-load all of setup before
            # the first tanh tile is produced.
            def emit_setup(m, narrow=False):
                # x2 chunk: bias-add + pair-duplication fused in one
                # PSUM-sourced op writing x2d[:, 2q+t] = x2[:, q] + b2.
                # narrow=True emits only batch-0's columns now (so the first
                # tanh can start early) and returns a closure for the rest.
                ps2 = psA.tile([128, BPC * N], f32, tag="psA", name=f"ps2_{m}")
                for k in range(KC):
                    nc.tensor.matmul(ps2[:, :], w2_sb[m][:, k * 128:(k + 1) * 128],
                                     xT_sb[k][:, :],
                                     start=(k == 0), stop=(k == KC - 1))
                psap = ps2[:, :]
                dst = x2d_sb[m][:, :]
                nw = N if narrow else BPC * N

                def dup(lo, n_):
                    in_ap = bass.AP(psap.tensor, psap.offset + lo,
                                    [[psap.ap[0][0], 128], [1, n_], [0, 2]])
                    out_ap = bass.AP(dst.tensor, dst.offset + 2 * lo,
                                     [[dst.ap[0][0], 128], [2, n_], [1, 2]])
                    nc.vector.tensor_scalar_add(out_ap, in_ap, b2_sb[:, m:m + 1])

                dup(0, nw)
                # x1 chunk
                ps1 = psA.tile([128, BPC * N], f32, tag="psA", name=f"ps1_{m}")
                for k in range(KC):
                    nc.tensor.matmul(ps1[:, :], w1_sb[m][:, k * 128:(k + 1) * 128],
                                     xT_sb[k][:, :],
                                     start=(k == 0), stop=(k == KC - 1))
                nc.vector.tensor_scalar_add(x1_sb[m][:, 0:nw], ps1[:, 0:nw],
                                            b1_sb[:, m:m + 1])
                if not narrow:
                    return None

                def rest():
                    dup(nw, BPC * N - nw)
                    nc.vector.tensor_scalar_add(x1_sb[m][:, nw:], ps1[:, nw:],
                                                b1_sb[:, m:m + 1])
                return rest

            # ---- main pipeline: DVE add -> ACT tanh -> PE reduce -> DMA out ----
            with (
                tc.tile_pool(name="psA", bufs=4, space=bass.MemorySpace.PSUM) as psA,
                tc.tile_pool(name="psO", bufs=4, space=bass.MemorySpace.PSUM) as psO,
            ):
                first = True
                rest_q = []
                for b in range(BPC):
                    # last batch ends with two half blocks so the post-ACT
                    # matmul/stage/DMA tail is shorter
                    blocks = [(k * IB, IB) for k in range(NIB)]
                    if b == BPC - 1:
                        i0L, nbL = blocks.pop()
                        blocks += [(i0L, nbL // 2), (i0L + nbL // 2, nbL // 2)]
                    for i0, nb in blocks:
                        fb = nb * N          # free elems this block
                        gh = fb // MMN // 2  # even/odd slice pairs
                        psE = psO.tile([GH, MMN], f32, tag="psO", name=f"psE_{b}_{i0}")
                        psF = psO.tile([GH, MMN], f32, tag="psO", name=f"psF_{b}_{i0}")
                        for c in range(MC):
                            # All chunks' setup is narrowed to batch-0 columns;
                            # each chunk's remainder runs two c-slots later so
                            # rests free psA slots just before the next chunk
                            # allocates, and the head-window DVE chain stays
                            # short (the rest columns aren't needed until b=1).
                            if first:
                                if c >= 2 and rest_q:
                                    rest_q.pop(0)()
                                rest_q.append(emit_setup(c, narrow=True))
                            elif b == 0 and i0 == IB and rest_q:
                                rest_q.pop(0)()
                            s = spool.tile([128, F], bf16, tag="s")
                            sap = s[:, :]
                            x1ap = x1_sb[c][:, b * N:(b + 1) * N]
                            in0 = bass.AP(x1ap.tensor, x1ap.offset,
                                          [[x1ap.ap[0][0], 128], [0, nb], [2, N // 2], [1, 2]])
                            x2ap = x2d_sb[c][:, :]
                            in1 = bass.AP(x2ap.tensor, x2ap.offset + (b * N + i0) * 2,
                                          [[x2ap.ap[0][0], 128], [2, nb], [0, N // 2], [1, 2]])
                            sout = bass.AP(sap.tensor, sap.offset,
                                           [[sap.ap[0][0], 128], [N, nb], [2, N // 2], [1, 2]])
                            tt = tpool.tile([128, F], bf16, tag="t")
                            if b == 0 and i0 == 0 and c <= 1:
                                # split the earliest tiles so the ACT engine
                                # starts sooner and stays fed
                                if c == 0:
                                    cuts = ((0, 16 * N), (16 * N, 40 * N), (40 * N, fb))
                                else:
                                    cuts = ((0, fb // 2), (fb // 2, fb))
                                for lo, hi in cuts:
                                    nbh = (hi - lo) // N
                                    in0h = bass.AP(in0.tensor, in0.offset,
                                                   [in0.ap[0], [0, nbh]] + in0.ap[2:])
                                    in1h = bass.AP(in1.tensor,
                                                   in1.offset + lo // N * 2,
                                                   [in1.ap[0], [2, nbh]] + in1.ap[2:])
                                    south = bass.AP(sout.tensor, sout.offset + lo,
                                                    [sout.ap[0], [N, nbh]] + sout.ap[2:])
                                    nc.vector.tensor_tensor(south, in0h, in1h,
                                                            mybir.AluOpType.add)
                                    nc.scalar.activation(tt[:, lo:hi], s[:, lo:hi],
                                                         AF.Tanh)
                            else:
                                nc.vector.tensor_tensor(sout, in0, in1,
                                                        mybir.AluOpType.add)
                                nc.scalar.activation(tt[:, :fb], s[:, :fb], AF.Tanh)
                            # slices 2r/2r+1 share lhsT (c, r) and go to the
                            # even/odd psum banks at partition r. On the very
                            # last (block, c) emit all E MMs first so stgE's
                            # staging/DMA overlaps the F MMs.
                            last_tail = (b == BPC - 1 and i0 == blocks[-1][0]
                                         and c == MC - 1)
                            order = ([(r, 0) for r in range(gh)]
                                     + [(r, 1) for r in range(gh)]) if last_tail                                 else [(r, p) for r in range(gh) for p in (0, 1)]
                            for r, p in order:
                                w0 = (c * GH + r) * GH
                                tgt = psE if p == 0 else psF
                                nc.tensor.matmul(tgt[:, :], wout_sb[:, w0:w0 + GH],
                                                 tt[:, (2 * r + p) * MMN:(2 * r + p + 1) * MMN],
                                                 start=(c == 0 and r == 0),
                                                 stop=(c == MC - 1 and r == gh - 1))
                        first = False
                        # stage PSUM->SBUF with +bout fused, then DMA out.
                        # stage row r of stgE/stgF = slice 2r / 2r+1.
                        stgE = stpool.tile([GH, MMN], f32, tag="stgE")
                        nc.vector.tensor_scalar_add(stgE[0:gh, :], psE[0:gh, :], boutf[0:gh, 0:1])
                        stgF = stpool.tile([GH, MMN], f32, tag="stgF")
                        nc.vector.tensor_scalar_add(stgF[0:gh, :], psF[0:gh, :], boutf[0:gh, 0:1])
                        o0 = i0 * N - N  # out-flat col of slice 0 (i=0 row dropped)
                        if i0 == 0:
                            # slice 0 = i rows 0..3; its first N cols are row i=0
                            nc.sync.dma_start(out[b:b + 1, 0:MMN - N], stgE[0:1, N:MMN])
                            ev = bass.AP(out[:, :].tensor,
                                         out[:, :].offset + b * (N - 1) * N + 2 * MMN - N,
                                         [[2 * MMN, gh - 1], [1, MMN]])
                            nc.sync.dma_start(ev, stgE[1:gh, :])
                        else:
                            ev = bass.AP(out[:, :].tensor,
                                         out[:, :].offset + b * (N - 1) * N + o0,
                                         [[2 * MMN, gh], [1, MMN]])
                            nc.sync.dma_start(ev, stgE[0:gh, :])
                        od = bass.AP(out[:, :].tensor,
                                     out[:, :].offset + b * (N - 1) * N + o0 + MMN,
                                     [[2 * MMN, gh], [1, MMN]])
                        nc.sync.dma_start(od, stgF[0:gh, :])

    nc.finalize()
    return nc


def _get_nc():
    if "nc" not in _CACHE:
        _CACHE["nc"] = _build_nc()
    return _CACHE["nc"]


def _prep_in_maps(x, W1, b1, W2, b2, Wout, bout):
    import ml_dtypes
    f = np.float32
    bf = ml_dtypes.bfloat16
    w1t = np.ascontiguousarray(
        np.asarray(W1, f).reshape(MC, 128, D).transpose(0, 2, 1).astype(bf))
    w2t = np.ascontiguousarray(
        np.asarray(W2, f).reshape(MC, 128, D).transpose(0, 2, 1).astype(bf))
    b1v = np.ascontiguousarray(np.asarray(b1, f).reshape(MC, 128).T)
    b2v = np.ascontiguousarray(np.asarray(b2, f).reshape(MC, 128).T)
    Wo = np.asarray(Wout, f)
    wop = np.zeros((128, MC * GH * GH), f)  # built f32, sent bf16
    for c in range(MC):
        for r in range(GH):
            wop[:, (c * GH + r) * GH + r] = Wo[c * 128:(c + 1) * 128]
    bov = np.full((128, 1), np.asarray(bout, f).reshape(()), f)
    x = np.asarray(x, f)
    in_maps = []
    for ci in range(NCORES):
        xs = x[ci * BPC:(ci + 1) * BPC]
        xTi = np.ascontiguousarray(
            xs.transpose(2, 0, 1).reshape(D, BPC * N).astype(bf))
        in_maps.append({
            "xT": xTi, "w1t": w1t, "w2t": w2t,
            "b1c": b1v, "b2c": b2v, "woutpad": wop.astype(bf), "bout": bov,
        })
    return in_maps


def _run(x, W1, b1, W2, b2, Wout, bout, trace=False):
    from concourse.bass_utils import run_bass_kernel_spmd

    nc = _get_nc()
    in_maps = _prep_in_maps(x, W1, b1, W2, b2, Wout, bout)
    res = run_bass_kernel_spmd(nc, in_maps, core_ids=list(range(NCORES)), trace=trace)
    outs = [np.asarray(res.results[ci]["out"]).reshape(BPC, N - 1, N)
            for ci in range(NCORES)]
    full = np.concatenate(outs, axis=0).astype(np.float32)
    return full, res


def kernel(x, W1, b1, W2, b2, Wout, bout):
    full, _ = _run(x, W1, b1, W2, b2, Wout, bout, trace=False)
    return full



# revision 4
# speedup vs baseline: 2.5455x; 2.5455x over previous
"""Trainium2 Bass kernel: AdditiveAttention scoring head via separable
Fourier-feature expansion.

Reference computes out[b,i-1,j] = sum_a Wout[a]*tanh(x1[b,j,a] + x2[b,i,a])
+ bout with x1 = x@W1.T + b1, x2 = x@W2.T + b2 (B=32, N=128, D=A=512).

Direct evaluation needs B*N*N*A = 268M tanh elements -> ACT-bound (~250us).
Instead approximate tanh(s) ~ sum_k b_k sin(w_k s) (K=10 freqs, two octave
lines {base*2^m}), so tanh(u+v) becomes sum_k b_k [sin_k(u)cos_k(v) +
cos_k(u)sin_k(v)] -- a rank-2K separable form. The NxN cross product then
collapses into TensorEngine matmuls over a (K,A) contraction and the
elementwise work drops to ~262K-element feature streams per core:

  ACT: base sin/cos per octave line (args |w0*y| <= pi/2, in Sin's valid
       range) and Square for sin^2 (both live in the trig_and_small table).
  DVE: frequency doubling s2=s*c (bf16 2x), cos via ts-dual 1-2*sigma^2,
       whose scalar slots also absorb Wout[a]*b_k weighting + cascade scale
       corrections; y staging PSUM->SBUF with bias add.
  PE : x1/x2 input matmuls + 320 accumulating [128p,127m,128n] matmuls
       contracting the feature dim into psum[b][i,j].

Sharding: data-parallel over batch across 8 cores (4 batches/core), weights
replicated, no collectives. Coefficients b_k fit offline (Gaussian-weighted
LS, ridge 1e-6); e2e rel err ~3.8e-3 (tolerance 2e-2).
"""
import sys
import numpy as np

if "/opt/trn_rl_repo" not in sys.path:
    sys.path.insert(0, "/opt/trn_rl_repo")

B, N, D, A = 32, 128, 512, 512
NCORES = 8
BPC = B // NCORES        # batches per core
TOK = BPC * N            # tokens per core (b,n flattened) = 512
KC = D // 128            # d contraction chunks
MC = A // 128            # a chunks
PI = float(np.pi)

# ---- offline Fourier fit of tanh: two octave lines ----
STRUCT = [(0.19, 5), (0.27, 5)]   # (base freq, levels); freqs base*2^m
RIDGE = 1e-6


def _fit_coeffs():
    sg = np.linspace(-11, 11, 4001)
    w = np.exp(-sg ** 2 / 4.0) + 1e-5
    t = np.tanh(sg)
    freqs = np.concatenate([[bb * 2 ** m for m in range(L)] for bb, L in STRUCT])
    X = np.sin(np.outer(sg, freqs))
    G = X.T @ (X * w[:, None])
    r = X.T @ (t * w)
    bk = np.linalg.solve(G + RIDGE * np.eye(len(freqs)), r)
    return bk.astype(np.float64)


_BK = _fit_coeffs()

_CACHE = {}


def _build_nc():
    import concourse.bass as bass
    import concourse.bacc as bacc
    import concourse.mybir as mybir
    from concourse import tile

    f32 = mybir.dt.float32
    bf16 = mybir.dt.bfloat16
    AF = mybir.ActivationFunctionType
    ALU = mybir.AluOpType

    nc = bacc.Bacc(None, target_bir_lowering=False)

    # DRAM params. xT: [d', k*TOK + t]; wpk: [d', ((side*MC+c)*KC+k)*128 + j]
    xT = nc.declare_dram_parameter("xT", [128, KC * TOK], bf16, isOutput=False)
    wpk = nc.declare_dram_parameter("wpk", [128, 2 * MC * KC * 128], bf16, isOutput=False)
    bias12 = nc.declare_dram_parameter("bias12", [128, 2 * MC], f32, isOutput=False)
    # wcoef columns: per pair k: m=0 -> [mul] per chunk (4 cols);
    # m>=1 -> [mul, add] per chunk (8 cols)
    NWC = sum((4 if m == 0 else 8) for bb, L in STRUCT for m in range(L))
    wcoef = nc.declare_dram_parameter("wcoef", [128, NWC], f32, isOutput=False)
    boutp = nc.declare_dram_parameter("bout", [128, 1], f32, isOutput=False)
    out = nc.declare_dram_parameter("out", [BPC, (N - 1) * N], f32, isOutput=True)

    with tile.TileContext(nc) as tc:
        with (
            tc.tile_pool(name="const", bufs=1) as cpool,
            tc.tile_pool(name="xw", bufs=1) as xwpool,
            tc.tile_pool(name="y", bufs=1) as ypool,
            tc.tile_pool(name="feat", bufs=1) as fpool,
            tc.tile_pool(name="stage", bufs=4) as stpool,
        ):
            # ---- PE warmup on junk while DMAs land ----
            warm = cpool.tile([128, 512], bf16, tag="warm")
            nc.gpsimd.memset(warm[:, :], 0.25)
            with tc.tile_pool(name="psW", bufs=1, space=bass.MemorySpace.PSUM) as psW:
                wps = psW.tile([128, 512], f32, tag="psW")
                for _ in range(9):
                    nc.tensor.matmul(wps[:, :], warm[:, 0:128], warm[:, :],
                                     start=True, stop=True)

            # ---- input DMAs across queues ----
            xsb = xwpool.tile([128, KC * TOK], bf16, tag="xsb")
            nc.sync.dma_start(xsb[:, 0:TOK * 2], xT[:, 0:TOK * 2])
            nc.scalar.dma_start(xsb[:, TOK * 2:], xT[:, TOK * 2:])
            wsb = xwpool.tile([128, 2 * MC * KC * 128], bf16, tag="wsb")
            H = MC * KC * 128
            nc.gpsimd.dma_start(wsb[:, 0:H], wpk[:, 0:H])
            nc.sync.dma_start(wsb[:, H:], wpk[:, H:])
            b12 = cpool.tile([128, 2 * MC], f32, tag="b12")
            nc.sync.dma_start(b12[:, :], bias12[:, :])
            wco = cpool.tile([128, NWC], f32, tag="wco")
            nc.sync.dma_start(wco[:, :], wcoef[:, :])
            boutf = cpool.tile([128, 1], f32, tag="boutf")
            nc.sync.dma_start(boutf[:, :], boutp[:, :])
            halfpi = cpool.tile([128, 1], f32, tag="halfpi")
            nc.gpsimd.memset(halfpi[:, :], PI / 2)

            def wslice(side, c, k):
                o = ((side * MC + c) * KC + k) * 128
                return wsb[:, o:o + 128]

            with (
                tc.tile_pool(name="psX", bufs=3, space=bass.MemorySpace.PSUM) as psX,
                tc.tile_pool(name="psO", bufs=4, space=bass.MemorySpace.PSUM) as psO,
            ):
                # ---- x1/x2 matmuls + staged (bias-added) y per side ----
                ys = []
                for side in range(2):
                    yt = ypool.tile([128, MC * TOK], f32, tag=f"y{side}",
                                    name=f"y{side}")
                    ys.append(yt)
                for side in range(2):
                    for c in range(MC):
                        ps = psX.tile([128, TOK], f32, tag="psX",
                                      name=f"psx_{side}_{c}")
                        for k in range(KC):
                            nc.tensor.matmul(ps[:, :], wslice(side, c, k),
                                             xsb[:, k * TOK:(k + 1) * TOK],
                                             start=(k == 0), stop=(k == KC - 1))
                        nc.vector.tensor_scalar(
                            ys[side][:, c * TOK:(c + 1) * TOK], ps[:, :],
                            b12[:, side * MC + c:side * MC + c + 1], None,
                            ALU.add)

                # ---- out psums ----
                pouts = [psO.tile([N - 1, N], f32, tag="psO", name=f"po{b}")
                         for b in range(BPC)]
                FW = MC * TOK  # feature tile width

                # cascade state per (line, side)
                nlines = len(STRUCT)
                st = {}
                mm_first = True

                def emit_mms(lw, m, c2w_t, s1_t, c1w_t, s2_t, last):
                    # A: psum[i,j] += sum_a c2w[a,i]*s1[a,j]
                    # B: psum[i,j] += sum_a s2[a,i]*c1w[a,j]
                    nonlocal mm_first
                    for c in range(MC):
                        for b in range(BPC):
                            t0 = c * TOK + b * N
                            st_ = mm_first and c == 0
                            for lhsT, rhs in ((c2w_t, s1_t), (s2_t, c1w_t)):
                                nc.tensor.matmul(
                                    pouts[b][:, :],
                                    lhsT[:, t0 + 1:t0 + N],
                                    rhs[:, t0:t0 + N],
                                    start=st_,
                                    stop=last and c == MC - 1 and
                                    lhsT is s2_t)
                                st_ = False
                    mm_first = False

                # wcoef column offsets per pair
                wc_off = []
                o = 0
                for li, (bb, L) in enumerate(STRUCT):
                    offs = []
                    for m in range(L):
                        offs.append(o)
                        o += 4 if m == 0 else 8
                    wc_off.append(offs)

                # ---- level 0: bases ----
                for li, (bb, L) in enumerate(STRUCT):
                    for side in range(2):
                        s0 = fpool.tile([128, FW], bf16, tag=f"s_{li}_{side}_p0",
                                        name=f"s{li}{side}0")
                        nc.scalar.activation(s0[:, :], ys[side][:, :], AF.Sin,
                                             bias=0.0, scale=float(bb))
                        c0 = fpool.tile([128, FW], bf16, tag=f"c_{li}_{side}_p0",
                                        name=f"c{li}{side}0")
                        nc.scalar.activation(c0[:, :], ys[side][:, :], AF.Sin,
                                             bias=halfpi[:, 0:1], scale=float(bb))
                        st[(li, side, 0)] = (s0, c0)
                # weighted cw0 + mms for level-0 pairs
                for li, (bb, L) in enumerate(STRUCT):
                    cws = []
                    for side in range(2):
                        s0, c0 = st[(li, side, 0)]
                        cw = fpool.tile([128, FW], bf16, tag=f"cw_{li}_{side}_p0",
                                        name=f"cw{li}{side}0")
                        for c in range(MC):
                            col = wc_off[li][0] + c
                            nc.vector.tensor_scalar(
                                cw[:, c * TOK:(c + 1) * TOK],
                                c0[:, c * TOK:(c + 1) * TOK],
                                wco[:, col:col + 1], None, ALU.mult)
                        cws.append(cw)
                    s1_t, _ = st[(li, 0, 0)]
                    s2_t, _ = st[(li, 1, 0)]
                    emit_mms(li, 0, cws[1], s1_t, cws[0], s2_t, last=False)

                # ---- levels >= 1, interleave the 4 chains ----
                Lmax = max(L for _, L in STRUCT)
                for m in range(1, Lmax):
                    # squares of level m-1 (ACT), round-robin chains
                    sqs = {}
                    for li, (bb, L) in enumerate(STRUCT):
                        if m >= L:
                            continue
                        for side in range(2):
                            sp, cp = st[(li, side, m - 1)]
                            sq = fpool.tile([128, FW], bf16,
                                            tag=f"sq_{li}_{side}",
                                            name=f"sq{li}{side}{m}")
                            nc.scalar.activation(sq[:, :], sp[:, :], AF.Square)
                            sqs[(li, side)] = sq
                    # DVE: s_m = s*c ; c_m (unweighted, if further cascade);
                    # cw_m (weighted)
                    for li, (bb, L) in enumerate(STRUCT):
                        if m >= L:
                            continue
                        lam_prev = 0.5 ** (m - 1)
                        for side in range(2):
                            sp, cp = st[(li, side, m - 1)]
                            sm = fpool.tile([128, FW], bf16,
                                            tag=f"s_{li}_{side}_p{m % 2}",
                                            name=f"s{li}{side}{m}")
                            nc.vector.tensor_tensor(sm[:, :], sp[:, :], cp[:, :],
                                                    ALU.mult)
                            cm = None
                            if m <= L - 2:
                                cm = fpool.tile([128, FW], bf16,
                                                tag=f"c_{li}_{side}_p{m % 2}",
                                                name=f"c{li}{side}{m}")
                                nc.vector.tensor_scalar(
                                    cm[:, :], sqs[(li, side)][:, :],
                                    float(-2.0 / lam_prev ** 2), 1.0,
                                    ALU.mult, ALU.add)
                            st[(li, side, m)] = (sm, cm)
                    for li, (bb, L) in enumerate(STRUCT):
                        if m >= L:
                            continue
                        cws = []
                        for side in range(2):
                            cw = fpool.tile([128, FW], bf16,
                                            tag=f"cw_{li}_{side}_p{m % 2}",
                                            name=f"cw{li}{side}{m}")
                            for c in range(MC):
                                col = wc_off[li][m] + 2 * c
                                nc.vector.tensor_scalar(
                                    cw[:, c * TOK:(c + 1) * TOK],
                                    sqs[(li, side)][:, c * TOK:(c + 1) * TOK],
                                    wco[:, col:col + 1],
                                    wco[:, col + 1:col + 2],
                                    ALU.mult, ALU.add)
                            cws.append(cw)
                        is_last = (m == Lmax - 1) and li == nlines - 1
                        s1_t = st[(li, 0, m)][0]
                        s2_t = st[(li, 1, m)][0]
                        emit_mms(li, m, cws[1], s1_t, cws[0], s2_t,
                                 last=is_last)

                # ---- stage + DMA out ----
                for b in range(BPC):
                    stg = stpool.tile([N - 1, N], f32, tag="stg")
                    nc.vector.tensor_scalar(stg[:, :], pouts[b][:, :],
                                            boutf[0:N - 1, 0:1], None, ALU.add)
                    oap = out[:, :]
                    dst = bass.AP(oap.tensor, oap.offset + b * (N - 1) * N,
                                  [[N, N - 1], [1, N]])
                    nc.sync.dma_start(dst, stg[:, :])

    nc.finalize()
    return nc


def _get_nc():
    if "nc" not in _CACHE:
        _CACHE["nc"] = _build_nc()
    return _CACHE["nc"]


def _prep_in_maps(x, W1, b1, W2, b2, Wout, bout):
    import ml_dtypes
    f = np.float32
    bfd = ml_dtypes.bfloat16
    x = np.asarray(x, f)
    W1 = np.asarray(W1, f)
    W2 = np.asarray(W2, f)
    Wo = np.asarray(Wout, f)

    # weights: wpk[d', ((side*MC+c)*KC+k)*128 + j] = Wside[c*128+j, k*128+d']
    def pack_w(W):
        # [MC, 128j, KC, 128d'] -> transpose to [128d', MC, KC, 128j]
        t = W.reshape(MC, 128, KC, 128).transpose(3, 0, 2, 1)
        return np.ascontiguousarray(t.reshape(128, MC * KC * 128))

    wpk = np.concatenate([pack_w(W1), pack_w(W2)], axis=1).astype(bfd)
    bias12 = np.ascontiguousarray(
        np.concatenate([np.asarray(b1, f).reshape(MC, 128).T,
                        np.asarray(b2, f).reshape(MC, 128).T], axis=1))

    # wcoef table
    NWC = sum((4 if m == 0 else 8) for bb, L in STRUCT for m in range(L))
    wco = np.zeros((128, NWC), f)
    o = 0
    ki = 0
    for bb, L in STRUCT:
        for m in range(L):
            lam = 0.5 ** m
            coef = _BK[ki + m] / lam
            if m == 0:
                for c in range(MC):
                    wco[:, o + c] = Wo[c * 128:(c + 1) * 128] * coef
                o += 4
            else:
                lam_prev = 0.5 ** (m - 1)
                for c in range(MC):
                    wchunk = Wo[c * 128:(c + 1) * 128]
                    wco[:, o + 2 * c] = wchunk * (-2.0 * coef / lam_prev ** 2)
                    wco[:, o + 2 * c + 1] = wchunk * coef
                o += 8
        ki += L
    bov = np.full((128, 1), np.asarray(bout, f).reshape(()), f)

    in_maps = []
    for ci in range(NCORES):
        xs = x[ci * BPC:(ci + 1) * BPC]          # [BPC, N, D]
        xt = xs.transpose(2, 0, 1).reshape(D, TOK)   # [D, TOK]
        # -> [128, KC*TOK]: xTi[d', k*TOK + t] = xt[k*128+d', t]
        xTi = np.ascontiguousarray(
            xt.reshape(KC, 128, TOK).transpose(1, 0, 2).reshape(128, KC * TOK)
        ).astype(bfd)
        in_maps.append({
            "xT": xTi, "wpk": wpk, "bias12": bias12,
            "wcoef": wco, "bout": bov,
        })
    return in_maps


def _run(x, W1, b1, W2, b2, Wout, bout, trace=False):
    from concourse.bass_utils import run_bass_kernel_spmd

    nc = _get_nc()
    in_maps = _prep_in_maps(x, W1, b1, W2, b2, Wout, bout)
    res = run_bass_kernel_spmd(nc, in_maps, core_ids=list(range(NCORES)),
                               trace=trace)
    outs = [np.asarray(res.results[ci]["out"]).reshape(BPC, N - 1, N)
            for ci in range(NCORES)]
    full = np.concatenate(outs, axis=0).astype(np.float32)
    return full, res


def kernel(x, W1, b1, W2, b2, Wout, bout):
    full, _ = _run(x, W1, b1, W2, b2, Wout, bout, trace=False)
    return full
